# revision 20
# baseline (speedup 1.0000x reference)
"""Trainium2 Bass kernel for soft decision-tree histogram binning.

Computes out[b, j] = prod_f softmax(x[b,f]*W + b_f, T=0.1)[digit_f(j)]
for x (4096, 7), cutpoints (7, 3) -> out (4096, 4**7=16384) float32.

Strategy (data-parallel over batch, 8 cores x 512 rows):
  - per-feature bias b_f from a 3-element min/mid/max sort of cutpoints,
    computed redundantly on all 128 partitions (no cross-partition traffic)
  - stabilized unnormalized e = exp((h - max_d h)/T) on the tiny (128, 28)
    tile; all 7 softmax denominators folded into one per-row scale
    C = 1/prod_f Z_f applied in the last cascade stage
  - output built as a Kronecker cascade (4 -> 16 -> 64 -> 256 -> 1024 via
    single double-broadcast tensor_tensor ops)
  - final scale ops write bf16 (fp32 compute, single rounding at the end,
    ~2^-9 max rel err), halving HBM write traffic to 16 MiB/core; the host
    upconverts to fp32 during the gather
  - the last two cascade levels' scalars are fused into a 16-entry per-row
    table sc16; output blocks come straight off the 1024-wide level:
    DVE blocks use one double-broadcast tensor_tensor per 4 KB-cols,
    Scalar blocks use 4x 1024-col activation-copy ops
  - DVE blocks stream out the Sync HWDGE queue, Scalar blocks the
    Activation HWDGE queue, so each DMA's HBM-write-receipt stall overlaps
    the other queue's data on the shared 16 SDMA engines
  - framework const-AP memsets are moved off the GpSimd(Pool) engine so
    the kernel-entry all-engine barrier does not wait for the slow Q7
    boot + library load (~5 us)
  - end-to-end HBM-write-drain bound: 16 MiB/core at ~358 GB/s
    => ~47 us stream + lead-in/teardown
"""

import numpy as np

B = 4096
F = 7
D1 = 4  # D+1 bins per feature
OUT = D1**F  # 16384
NCORES = 8
ROWS = B // NCORES  # 512
P = 128
NTILES = ROWS // P  # 4
INV_T = 10.0

_cache = {}


def _build_bass():
    import concourse.bacc as bacc
    import concourse.tile as tile
    from concourse import mybir

    f32 = mybir.dt.float32
    bf16 = mybir.dt.bfloat16
    Alu = mybir.AluOpType
    Act = mybir.ActivationFunctionType
    AX = mybir.AxisListType.X

    from concourse.vector_clock import ScopedClock

    class LeanTileContext(tile.TileContext):
        """TileContext with a minimal kernel exit: keep the sync-engine
        drain that waits for all outstanding work (so the NEFF cannot
        complete with DMAs in flight), skip the two all-engine barriers
        and the semaphore recycle loop. Each kernel() call compiles and
        loads a fresh NEFF, so semaphores never need to be handed back."""

        def _drain_and_barrier(self, tick_clock, wait_clock):
            drain_inst = self.nc.sync.drain()
            wait_clock.add_sem_waits(
                drain_inst.ins, ScopedClock({None: tick_clock.global_clock})
            )
            popped = self.nc._tile_sem_poison_stack.pop()
            assert popped is self._sem_poison

    nc = bacc.Bacc("TRN2", target_bir_lowering=False, debug=False)

    # The framework preamble emits 4 const-AP memsets on the Pool engine;
    # any Pool compute instruction drags in the GpSimd Q7 boot + library
    # load (~5 us) which the all-engine entry barrier then waits on. Move
    # them to DVE (which also supports memset) so Pool's stream is only
    # drain/event/branch on the fast NX sequencer.
    for ins in nc.main_func.blocks[0].instructions:
        if isinstance(ins, mybir.InstMemset) and ins.engine == mybir.EngineType.Pool:
            ins.engine = mybir.EngineType.DVE

    # xw[p, :] = [x tile0 (7) | W pattern (28) | cutpoints (21) | x tiles 1-3 (21)]
    # critical prefix (56 cols) loads in a first DMA so tile 0 can start
    # before the rest of x lands
    NCRIT = F + F * D1 + F * 3  # 56
    XWC = NCRIT + (NTILES - 1) * F  # 77
    xw_d = nc.dram_tensor("xw", [P, XWC], f32, kind="ExternalInput").ap()
    out_d = nc.dram_tensor("out", [ROWS, OUT], bf16, kind="ExternalOutput").ap()

    with LeanTileContext(nc) as tc:
        with (
            tc.tile_pool(name="const", bufs=1) as cpool,
            tc.tile_pool(name="small", bufs=2) as sp,
            tc.tile_pool(name="epool", bufs=4) as ep,
            tc.tile_pool(name="scpool", bufs=3) as scp,
            tc.tile_pool(name="mid", bufs=3) as mp,
            tc.tile_pool(name="blk", bufs=5) as blkp,
            tc.tile_pool(name="fblk", bufs=2) as fbp,
        ):
            # input DMAs: critical prefix first, rest of x second
            xw = cpool.tile([P, XWC], f32)
            nc.sync.dma_start(out=xw[:, 0:NCRIT], in_=xw_d[:, 0:NCRIT])
            nc.sync.dma_start(out=xw[:, NCRIT:], in_=xw_d[:, NCRIT:])
            w4 = xw[:, F : F + F * D1].rearrange("p (f d) -> p f d", d=D1)
            cp3 = xw[:, F + F * D1 : NCRIT].rearrange("p (f c) -> p f c", c=3)

            def xt_of(t):
                return (
                    xw[:, 0:F]
                    if t == 0
                    else xw[:, NCRIT + (t - 1) * F : NCRIT + t * F]
                )

            # b_f = [0, -min, max-sum, -sum] per feature (cumsum of -sorted cuts)
            vmax = cpool.tile([P, F], f32)
            brep = cpool.tile([P, F * D1], f32)
            b4 = brep.rearrange("p (f d) -> p f d", d=D1)
            nc.vector.memset(b4[:, :, 0], 0.0)
            nc.vector.tensor_reduce(out=b4[:, :, 1], in_=cp3, axis=AX, op=Alu.min, negate=True)
            nc.vector.tensor_reduce(out=b4[:, :, 3], in_=cp3, axis=AX, op=Alu.add, negate=True)
            nc.vector.tensor_reduce(out=vmax, in_=cp3, axis=AX, op=Alu.max)
            nc.vector.tensor_tensor(out=b4[:, :, 2], in0=vmax, in1=b4[:, :, 3], op=Alu.add)

            es = [None] * NTILES
            sc16s = [None] * NTILES
            t5s = [None] * NTILES

            def hchain(t):
                # h[p, f, d] = x[p,f]*W[d] + b[f,d]; stabilize; e = exp(h/T)
                xt = xt_of(t)
                h = sp.tile([P, F * D1], f32, tag="h")
                h4 = h.rearrange("p (f d) -> p f d", d=D1)
                xb = xt[:, :, None].broadcast_to((P, F, D1))
                nc.vector.tensor_tensor(out=h4, in0=xb, in1=w4, op=Alu.mult)
                nc.vector.tensor_tensor(out=h4, in0=h4, in1=b4, op=Alu.add)
                m7 = sp.tile([P, F], f32, tag="m7")
                nc.vector.tensor_reduce(out=m7, in_=h4, axis=AX, op=Alu.max)
                mb = m7[:, :, None].broadcast_to((P, F, D1))
                nc.vector.tensor_tensor(out=h4, in0=h4, in1=mb, op=Alu.subtract)
                e = ep.tile([P, F * D1], f32, tag="e")
                nc.scalar.activation(out=e, in_=h, func=Act.Exp, scale=INV_T)
                es[t] = e

            def zchain(t):
                # C = 1 / prod_f Z_f; sc16[d1*4+d0] = e[f1,d1] * e[f0,d0] * C
                e = es[t]
                e4 = e.rearrange("p (f d) -> p f d", d=D1)
                z7 = sp.tile([P, F], f32, tag="z7")
                nc.vector.tensor_reduce(out=z7, in_=e4, axis=AX, op=Alu.add)
                zp = sp.tile([P, 1], f32, tag="zp")
                nc.vector.tensor_reduce(out=zp, in_=z7, axis=AX, op=Alu.mult)
                c1 = sp.tile([P, 1], f32, tag="c1")
                nc.vector.reciprocal(out=c1, in_=zp)
                sc = sp.tile([P, D1], f32, tag="sc")
                nc.vector.tensor_scalar_mul(out=sc, in0=e[:, 0:D1], scalar1=c1)
                sc16 = scp.tile([P, 16], f32, tag="sc16")
                nc.vector.tensor_tensor(
                    out=sc16.rearrange("p (a b) -> p a b", b=D1),
                    in0=e[:, 4:8, None].broadcast_to((P, D1, D1)),
                    in1=sc[:, None, :].broadcast_to((P, D1, D1)),
                    op=Alu.mult,
                )
                sc16s[t] = sc16

            def cascade(t):
                # Kronecker cascade over features 6,5 -> ... -> 2: 16->64->256->1024
                e = es[t]
                t2 = sp.tile([P, 16], f32, tag="t2")
                nc.vector.tensor_tensor(
                    out=t2.rearrange("p (a b) -> p a b", b=D1),
                    in0=e[:, 20:24, None].broadcast_to((P, D1, D1)),
                    in1=e[:, None, 24:28].broadcast_to((P, D1, D1)),
                    op=Alu.mult,
                )
                t3 = sp.tile([P, 64], f32, tag="t3")
                nc.vector.tensor_tensor(
                    out=t3.rearrange("p (a b) -> p a b", b=16),
                    in0=e[:, 16:20, None].broadcast_to((P, D1, 16)),
                    in1=t2[:, None, :].broadcast_to((P, D1, 16)),
                    op=Alu.mult,
                )
                t4 = sp.tile([P, 256], f32, tag="t4")
                nc.vector.tensor_tensor(
                    out=t4.rearrange("p (a b) -> p a b", b=64),
                    in0=e[:, 12:16, None].broadcast_to((P, D1, 64)),
                    in1=t3[:, None, :].broadcast_to((P, D1, 64)),
                    op=Alu.mult,
                )
                t5 = mp.tile([P, 1024], f32, tag="t5")
                for d in range(D1):
                    nc.vector.tensor_scalar_mul(
                        out=t5[:, d * 256 : (d + 1) * 256],
                        in0=t4,
                        scalar1=e[:, 8 + d : 9 + d],
                    )
                t5s[t] = t5

            def scale_cols(t, base, nsub, blk, off, eng):
                # nsub x 1024-col scale ops into blk at col offset off
                t5, sc16 = t5s[t], sc16s[t]
                for s in range(nsub):
                    d0, d1 = (base + s) // D1, (base + s) % D1
                    scol = sc16[:, d1 * D1 + d0 : d1 * D1 + d0 + 1]
                    q = blk[:, (off + s) * 1024 : (off + s + 1) * 1024]
                    if eng == "v":
                        nc.vector.tensor_scalar_mul(out=q, in0=t5, scalar1=scol)
                    else:
                        nc.scalar.mul(out=q, in_=t5, mul=scol)

            def lead_block(t, base, nsub, eng, q):
                # small standalone block to get the stream started early
                rows = slice(t * P, (t + 1) * P)
                blk = blkp.tile([P, nsub * 1024], bf16, tag="blk")
                scale_cols(t, base, nsub, blk, 0, eng)
                q.dma_start(
                    out=out_d[rows, base * 1024 : (base + nsub) * 1024], in_=blk
                )

            def half_block(t, base, nsub, eng, q):
                # 8 KB-col half-tile block: DVE halves stream out the Sync
                # HWDGE queue, Scalar halves out the Act queue; big DMAs
                # keep HBM-write-receipt stalls on SDMA engine 15 rare
                rows = slice(t * P, (t + 1) * P)
                blk = fbp.tile([P, nsub * 1024], bf16, tag="fblk")
                scale_cols(t, base, nsub, blk, 0, eng)
                q.dma_start(
                    out=out_d[rows, base * 1024 : (base + nsub) * 1024], in_=blk
                )

            # tile 0 leads with small DVE blocks so the stream starts early
            hchain(0)
            zchain(0)
            cascade(0)
            lead_block(0, 0, 1, "v", nc.sync)
            lead_block(0, 1, 1, "v", nc.sync)
            lead_block(0, 2, 2, "v", nc.sync)
            # exp for the other tiles up front so Scalar's big block runs
            # never gate the next tile's DVE cascade
            hchain(1)
            hchain(2)
            hchain(3)
            lead_block(0, 4, 4, "v", nc.sync)
            half_block(0, 8, 8, "s", nc.scalar)
            for t in range(1, NTILES):
                zchain(t)
                cascade(t)
                half_block(t, 0, 8, "v", nc.sync)
                if t < NTILES - 1:
                    half_block(t, 8, 8, "s", nc.scalar)
                else:
                    # taper the last tile's Scalar blocks so the final DMAs
                    # are small and the drain tail collapses
                    lead_block(t, 8, 4, "s", nc.scalar)
                    lead_block(t, 12, 2, "s", nc.scalar)
                    lead_block(t, 14, 1, "s", nc.scalar)
                    lead_block(t, 15, 1, "s", nc.scalar)

    # Hoist the input DMA ahead of the kernel-entry all-engine barrier: the
    # SP engine's instruction stream is ready almost immediately while
    # DVE/Act boot ~6-7 us later (serial istream fetch), so issuing the
    # input load first overlaps its descriptor-gen + HBM latency with the
    # other engines' boots. Safe only because the DMA has no waits and its
    # completion-semaphore updates move with it.
    main_bb = nc.main_func.blocks[0]
    tile_bb = next(b for b in nc.main_func.blocks if b.name.startswith("tile_context"))
    in_dmas = [
        ins
        for ins in tile_bb.instructions
        if isinstance(ins, mybir.InstDMACopy) and ins.engine == mybir.EngineType.SP
    ][:2]
    pos = 1
    for in_dma in in_dmas:
        si = in_dma.sync_info
        if si is None or not si.on_wait:
            tile_bb.instructions.remove(in_dma)
            main_bb.instructions.insert(pos, in_dma)
            pos += 1

    # Drop the kernel-entry all-engine barrier (per-engine InstDrain +
    # InstEventSemaphore): it only orders the const-AP memsets before
    # their first use, but the sole const consumer here (Exp's 0.0 bias
    # on the Act engine) is already transitively ordered behind DVE's
    # memsets via its semaphore wait on the DVE-produced h tile. Without
    # the barrier no engine waits for the slowest engine's ~7 us
    # instruction-stream boot before starting real work.
    main_bb.instructions[:] = [
        ins
        for ins in main_bb.instructions
        if not isinstance(ins, (mybir.InstDrain, mybir.InstEventSemaphore))
    ]

    nc.compile()
    return nc


def build_in_maps(x, cutpoints):
    # layout: [x tile0 (7) | W pattern (28) | cutpoints (21) | x tiles 1-3 (21)]
    NCRIT = F + F * D1 + F * 3
    XWC = NCRIT + (NTILES - 1) * F
    wpat = np.tile(np.arange(1.0, D1 + 1.0, dtype=np.float32), F)
    cflat = cutpoints.ravel().astype(np.float32)
    # x sharded: core k, partition p gets rows k*512 + {p, 128+p, 256+p, 384+p}
    xs = (
        x.reshape(NCORES, NTILES, P, F)
        .transpose(0, 2, 1, 3)
        .reshape(NCORES, P, NTILES * F)
    )
    in_maps = []
    for k in range(NCORES):
        xw = np.empty((P, XWC), dtype=np.float32)
        xw[:, 0:F] = xs[k][:, 0:F]
        xw[:, F : F + F * D1] = wpat
        xw[:, F + F * D1 : NCRIT] = cflat
        xw[:, NCRIT:] = xs[k][:, F:]
        in_maps.append({"xw": xw})
    return in_maps


def kernel(x, cutpoints):
    from concourse import bass_utils

    if "nc" not in _cache:
        _cache["nc"] = _build_bass()
    nc = _cache["nc"]

    x = np.ascontiguousarray(np.asarray(x), dtype=np.float32)
    cutpoints = np.ascontiguousarray(np.asarray(cutpoints), dtype=np.float32)
    in_maps = build_in_maps(x, cutpoints)
    res = bass_utils.run_bass_kernel_spmd(nc, in_maps, list(range(NCORES))).results
    return np.concatenate(
        [np.asarray(res[k]["out"]).astype(np.float32) for k in range(NCORES)], axis=0
    )


# revision 21
# speedup vs baseline: 1.2837x; 1.2837x over previous
"""Trainium2 Bass kernel for soft decision-tree histogram binning.

Computes out[b, j] = prod_f softmax(x[b,f]*W + b_f, T=0.1)[digit_f(j)]
for x (4096, 7), cutpoints (7, 3) -> out (4096, 4**7=16384) float32.

Strategy (data-parallel over batch, 8 cores x 512 rows):
  - per-feature bias b_f from a 3-element min/mid/max sort of cutpoints,
    computed redundantly on all 128 partitions (no cross-partition traffic)
  - stabilized unnormalized e = exp((h - max_d h)/T) on the tiny (128, 28)
    tile; all 7 softmax denominators folded into one per-row scale
    C = 1/prod_f Z_f applied in the last cascade stage
  - output built as a Kronecker cascade (4 -> 16 -> 64 -> 256 -> 1024 via
    single double-broadcast tensor_tensor ops)
  - final scale ops write bf16 (fp32 compute, single rounding at the end,
    ~2^-9 max rel err), halving HBM write traffic to 16 MiB/core; the host
    upconverts to fp32 during the gather
  - the last two cascade levels' scalars are fused into a 16-entry per-row
    table sc16; output blocks come straight off the 1024-wide level:
    DVE blocks use one double-broadcast tensor_tensor per 4 KB-cols,
    Scalar blocks use 4x 1024-col activation-copy ops
  - DVE blocks stream out the Sync HWDGE queue, Scalar blocks the
    Activation HWDGE queue, so each DMA's HBM-write-receipt stall overlaps
    the other queue's data on the shared 16 SDMA engines
  - framework const-AP memsets are moved off the GpSimd(Pool) engine so
    the kernel-entry all-engine barrier does not wait for the slow Q7
    boot + library load (~5 us)
  - end-to-end HBM-write-drain bound: 16 MiB/core at ~358 GB/s
    => ~47 us stream + lead-in/teardown
"""

import numpy as np

B = 4096
F = 7
D1 = 4  # D+1 bins per feature
OUT = D1**F  # 16384
NCORES = 8
ROWS = B // NCORES  # 512
P = 128
NTILES = ROWS // P  # 4
INV_T = 10.0

_cache = {}


def _build_bass():
    import concourse.bacc as bacc
    import concourse.tile as tile
    from concourse import mybir

    f32 = mybir.dt.float32
    bf16 = mybir.dt.bfloat16
    Alu = mybir.AluOpType
    Act = mybir.ActivationFunctionType
    AX = mybir.AxisListType.X

    from concourse.vector_clock import ScopedClock

    class LeanTileContext(tile.TileContext):
        """TileContext with a minimal kernel exit: keep the sync-engine
        drain that waits for all outstanding work (so the NEFF cannot
        complete with DMAs in flight), skip the two all-engine barriers
        and the semaphore recycle loop. Each kernel() call compiles and
        loads a fresh NEFF, so semaphores never need to be handed back."""

        def _drain_and_barrier(self, tick_clock, wait_clock):
            drain_inst = self.nc.sync.drain()
            wait_clock.add_sem_waits(
                drain_inst.ins, ScopedClock({None: tick_clock.global_clock})
            )
            popped = self.nc._tile_sem_poison_stack.pop()
            assert popped is self._sem_poison

    nc = bacc.Bacc("TRN2", target_bir_lowering=False, debug=False)

    # The framework preamble emits 4 const-AP memsets on the Pool engine;
    # any Pool compute instruction drags in the GpSimd Q7 boot + library
    # load (~5 us) which the all-engine entry barrier then waits on. Move
    # them to DVE (which also supports memset) so Pool's stream is only
    # drain/event/branch on the fast NX sequencer.
    for ins in nc.main_func.blocks[0].instructions:
        if isinstance(ins, mybir.InstMemset) and ins.engine == mybir.EngineType.Pool:
            ins.engine = mybir.EngineType.DVE

    # xw[p, :] = [x tile0 (7) | W pattern (28) | cutpoints (21) | x tiles 1-3 (21)]
    # critical prefix (56 cols) loads in a first DMA so tile 0 can start
    # before the rest of x lands
    NCRIT = F + F * D1 + F * 3  # 56
    XWC = NCRIT + (NTILES - 1) * F  # 77
    xw_d = nc.dram_tensor("xw", [P, XWC], f32, kind="ExternalInput").ap()
    out_d = nc.dram_tensor("out", [ROWS, OUT], bf16, kind="ExternalOutput").ap()

    with LeanTileContext(nc) as tc:
        with (
            tc.tile_pool(name="const", bufs=1) as cpool,
            tc.tile_pool(name="small", bufs=2) as sp,
            tc.tile_pool(name="epool", bufs=4) as ep,
            tc.tile_pool(name="scpool", bufs=3) as scp,
            tc.tile_pool(name="mid", bufs=3) as mp,
            tc.tile_pool(name="blk", bufs=5) as blkp,
            tc.tile_pool(name="fblk", bufs=4) as fbp,
        ):
            # input DMAs: critical prefix first, rest of x second
            xw = cpool.tile([P, XWC], f32)
            nc.sync.dma_start(out=xw[:, 0:NCRIT], in_=xw_d[:, 0:NCRIT])
            nc.sync.dma_start(out=xw[:, NCRIT:], in_=xw_d[:, NCRIT:])
            w4 = xw[:, F : F + F * D1].rearrange("p (f d) -> p f d", d=D1)
            cp3 = xw[:, F + F * D1 : NCRIT].rearrange("p (f c) -> p f c", c=3)

            def xt_of(t):
                return (
                    xw[:, 0:F]
                    if t == 0
                    else xw[:, NCRIT + (t - 1) * F : NCRIT + t * F]
                )

            # b_f = [0, -min, max-sum, -sum] per feature (cumsum of -sorted cuts)
            vmax = cpool.tile([P, F], f32)
            brep = cpool.tile([P, F * D1], f32)
            b4 = brep.rearrange("p (f d) -> p f d", d=D1)
            nc.vector.memset(b4[:, :, 0], 0.0)
            nc.vector.tensor_reduce(out=b4[:, :, 1], in_=cp3, axis=AX, op=Alu.min, negate=True)
            nc.vector.tensor_reduce(out=b4[:, :, 3], in_=cp3, axis=AX, op=Alu.add, negate=True)
            nc.vector.tensor_reduce(out=vmax, in_=cp3, axis=AX, op=Alu.max)
            nc.vector.tensor_tensor(out=b4[:, :, 2], in0=vmax, in1=b4[:, :, 3], op=Alu.add)

            es = [None] * NTILES
            sc16s = [None] * NTILES
            t5s = [None] * NTILES

            def hchain(t):
                # h[p, f, d] = x[p,f]*W[d] + b[f,d]; stabilize; e = exp(h/T)
                xt = xt_of(t)
                h = sp.tile([P, F * D1], f32, tag="h")
                h4 = h.rearrange("p (f d) -> p f d", d=D1)
                xb = xt[:, :, None].broadcast_to((P, F, D1))
                nc.vector.tensor_tensor(out=h4, in0=xb, in1=w4, op=Alu.mult)
                nc.vector.tensor_tensor(out=h4, in0=h4, in1=b4, op=Alu.add)
                m7 = sp.tile([P, F], f32, tag="m7")
                nc.vector.tensor_reduce(out=m7, in_=h4, axis=AX, op=Alu.max)
                mb = m7[:, :, None].broadcast_to((P, F, D1))
                nc.vector.tensor_tensor(out=h4, in0=h4, in1=mb, op=Alu.subtract)
                e = ep.tile([P, F * D1], f32, tag="e")
                nc.scalar.activation(out=e, in_=h, func=Act.Exp, scale=INV_T)
                es[t] = e

            def zchain(t):
                # C = 1 / prod_f Z_f; sc16[d1*4+d0] = e[f1,d1] * e[f0,d0] * C
                e = es[t]
                e4 = e.rearrange("p (f d) -> p f d", d=D1)
                z7 = sp.tile([P, F], f32, tag="z7")
                nc.vector.tensor_reduce(out=z7, in_=e4, axis=AX, op=Alu.add)
                zp = sp.tile([P, 1], f32, tag="zp")
                nc.vector.tensor_reduce(out=zp, in_=z7, axis=AX, op=Alu.mult)
                c1 = sp.tile([P, 1], f32, tag="c1")
                nc.vector.reciprocal(out=c1, in_=zp)
                sc = sp.tile([P, D1], f32, tag="sc")
                nc.vector.tensor_scalar_mul(out=sc, in0=e[:, 0:D1], scalar1=c1)
                sc16 = scp.tile([P, 16], f32, tag="sc16")
                nc.vector.tensor_tensor(
                    out=sc16.rearrange("p (a b) -> p a b", b=D1),
                    in0=e[:, 4:8, None].broadcast_to((P, D1, D1)),
                    in1=sc[:, None, :].broadcast_to((P, D1, D1)),
                    op=Alu.mult,
                )
                sc16s[t] = sc16

            def cascade(t):
                # Kronecker cascade over features 6,5 -> ... -> 2: 16->64->256->1024
                e = es[t]
                t2 = sp.tile([P, 16], f32, tag="t2")
                nc.vector.tensor_tensor(
                    out=t2.rearrange("p (a b) -> p a b", b=D1),
                    in0=e[:, 20:24, None].broadcast_to((P, D1, D1)),
                    in1=e[:, None, 24:28].broadcast_to((P, D1, D1)),
                    op=Alu.mult,
                )
                t3 = sp.tile([P, 64], f32, tag="t3")
                nc.vector.tensor_tensor(
                    out=t3.rearrange("p (a b) -> p a b", b=16),
                    in0=e[:, 16:20, None].broadcast_to((P, D1, 16)),
                    in1=t2[:, None, :].broadcast_to((P, D1, 16)),
                    op=Alu.mult,
                )
                t4 = sp.tile([P, 256], f32, tag="t4")
                nc.vector.tensor_tensor(
                    out=t4.rearrange("p (a b) -> p a b", b=64),
                    in0=e[:, 12:16, None].broadcast_to((P, D1, 64)),
                    in1=t3[:, None, :].broadcast_to((P, D1, 64)),
                    op=Alu.mult,
                )
                t5 = mp.tile([P, 1024], f32, tag="t5")
                for d in range(D1):
                    nc.vector.tensor_scalar_mul(
                        out=t5[:, d * 256 : (d + 1) * 256],
                        in0=t4,
                        scalar1=e[:, 8 + d : 9 + d],
                    )
                t5s[t] = t5

            def scale_cols(t, base, nsub, blk, off, eng):
                # nsub x 1024-col scale ops into blk at col offset off
                t5, sc16 = t5s[t], sc16s[t]
                for s in range(nsub):
                    d0, d1 = (base + s) // D1, (base + s) % D1
                    scol = sc16[:, d1 * D1 + d0 : d1 * D1 + d0 + 1]
                    q = blk[:, (off + s) * 1024 : (off + s + 1) * 1024]
                    if eng == "v":
                        nc.vector.tensor_scalar_mul(out=q, in0=t5, scalar1=scol)
                    else:
                        nc.scalar.mul(out=q, in_=t5, mul=scol)

            def lead_block(t, base, nsub, eng, q):
                # small standalone block to get the stream started early
                rows = slice(t * P, (t + 1) * P)
                blk = blkp.tile([P, nsub * 1024], bf16, tag="blk")
                scale_cols(t, base, nsub, blk, 0, eng)
                q.dma_start(
                    out=out_d[rows, base * 1024 : (base + nsub) * 1024], in_=blk
                )

            def half_block(t, base, nsub, eng, q):
                # 8 KB-col half-tile block: DVE halves stream out the Sync
                # HWDGE queue, Scalar halves out the Act queue; big DMAs
                # keep HBM-write-receipt stalls on SDMA engine 15 rare
                rows = slice(t * P, (t + 1) * P)
                blk = fbp.tile([P, nsub * 1024], bf16, tag="fblk")
                scale_cols(t, base, nsub, blk, 0, eng)
                q.dma_start(
                    out=out_d[rows, base * 1024 : (base + nsub) * 1024], in_=blk
                )

            # tile 0 leads with small DVE blocks so the stream starts early
            hchain(0)
            zchain(0)
            cascade(0)
            lead_block(0, 0, 1, "v", nc.sync)
            lead_block(0, 1, 1, "v", nc.sync)
            lead_block(0, 2, 2, "v", nc.sync)
            # exp for the other tiles up front so Scalar's big block runs
            # never gate the next tile's DVE cascade
            hchain(1)
            hchain(2)
            hchain(3)
            lead_block(0, 4, 4, "v", nc.sync)
            half_block(0, 8, 8, "s", nc.scalar)
            for t in range(1, NTILES):
                zchain(t)
                cascade(t)
                half_block(t, 0, 8, "v", nc.sync)
                if t < NTILES - 1:
                    half_block(t, 8, 8, "s", nc.scalar)
                else:
                    # taper the last tile's Scalar blocks so the final DMAs
                    # are small and the drain tail collapses
                    lead_block(t, 8, 4, "s", nc.scalar)
                    lead_block(t, 12, 2, "s", nc.scalar)
                    lead_block(t, 14, 1, "s", nc.scalar)
                    lead_block(t, 15, 1, "s", nc.scalar)

    # Hoist the input DMA ahead of the kernel-entry all-engine barrier: the
    # SP engine's instruction stream is ready almost immediately while
    # DVE/Act boot ~6-7 us later (serial istream fetch), so issuing the
    # input load first overlaps its descriptor-gen + HBM latency with the
    # other engines' boots. Safe only because the DMA has no waits and its
    # completion-semaphore updates move with it.
    main_bb = nc.main_func.blocks[0]
    tile_bb = next(b for b in nc.main_func.blocks if b.name.startswith("tile_context"))
    in_dmas = [
        ins
        for ins in tile_bb.instructions
        if isinstance(ins, mybir.InstDMACopy) and ins.engine == mybir.EngineType.SP
    ][:2]
    pos = 1
    for in_dma in in_dmas:
        si = in_dma.sync_info
        if si is None or not si.on_wait:
            tile_bb.instructions.remove(in_dma)
            main_bb.instructions.insert(pos, in_dma)
            pos += 1

    # Drop the kernel-entry all-engine barrier (per-engine InstDrain +
    # InstEventSemaphore): it only orders the const-AP memsets before
    # their first use, but the sole const consumer here (Exp's 0.0 bias
    # on the Act engine) is already transitively ordered behind DVE's
    # memsets via its semaphore wait on the DVE-produced h tile. Without
    # the barrier no engine waits for the slowest engine's ~7 us
    # instruction-stream boot before starting real work.
    main_bb.instructions[:] = [
        ins
        for ins in main_bb.instructions
        if not isinstance(ins, (mybir.InstDrain, mybir.InstEventSemaphore))
    ]

    nc.compile()
    return nc


def build_in_maps(x, cutpoints):
    # layout: [x tile0 (7) | W pattern (28) | cutpoints (21) | x tiles 1-3 (21)]
    NCRIT = F + F * D1 + F * 3
    XWC = NCRIT + (NTILES - 1) * F
    wpat = np.tile(np.arange(1.0, D1 + 1.0, dtype=np.float32), F)
    cflat = cutpoints.ravel().astype(np.float32)
    # x sharded: core k, partition p gets rows k*512 + {p, 128+p, 256+p, 384+p}
    xs = (
        x.reshape(NCORES, NTILES, P, F)
        .transpose(0, 2, 1, 3)
        .reshape(NCORES, P, NTILES * F)
    )
    in_maps = []
    for k in range(NCORES):
        xw = np.empty((P, XWC), dtype=np.float32)
        xw[:, 0:F] = xs[k][:, 0:F]
        xw[:, F : F + F * D1] = wpat
        xw[:, F + F * D1 : NCRIT] = cflat
        xw[:, NCRIT:] = xs[k][:, F:]
        in_maps.append({"xw": xw})
    return in_maps


def kernel(x, cutpoints):
    from concourse import bass_utils

    if "nc" not in _cache:
        _cache["nc"] = _build_bass()
    nc = _cache["nc"]

    x = np.ascontiguousarray(np.asarray(x), dtype=np.float32)
    cutpoints = np.ascontiguousarray(np.asarray(cutpoints), dtype=np.float32)
    in_maps = build_in_maps(x, cutpoints)
    res = bass_utils.run_bass_kernel_spmd(nc, in_maps, list(range(NCORES))).results
    return np.concatenate(
        [np.asarray(res[k]["out"]).astype(np.float32) for k in range(NCORES)], axis=0
    )
